# revision 18
# baseline (speedup 1.0000x reference)
"""Distributed Trainium2 Bass kernel for the associative-embedding (AE) loss.

Problem: per image b (B=8), two tag maps (tm0 [J,256,256], tm1 [J,512,512]),
keypoints kps [NH, 3*J] (x, y, vis interleaved, NH=30 humans, J=17 joints).
Per level: gather tag values at (j, x, y), masked per-human mean, pull loss
(masked squared deviation / num_humans) + push loss (pairwise Gaussian of
means / num_humans^2).  Output: per-image loss [B] (sum over both levels).

Strategy: pure data-parallel over B across 8 NeuronCores (core b handles
image b).  The loss touches only NH*J = 510 elements of each tag map, so
instead of streaming the 178 MB of tag maps, each core computes flat gather
indices on-chip from the keypoint data and pulls exactly 1020 scalars out
of DRAM via 8 indirect (SWDGE) DMAs of 128 single-element descriptors each
(HW indirect DMA = one descriptor per out partition row).  The gathered
values live in a [128, 8] chunk layout; one-hot matrices passed from the
host let the tensor engine reduce that layout directly into per-human
sufficient statistics (sum of masked vals, sum of masked vals^2), since
pull = sum(m v^2) - sv*avg.  Per-chunk stat products and matmuls are
pipelined under the remaining gathers.  The push loss uses a 32x32 DVE
stream transpose for the pairwise mean differences.  Per-core output is a
single scalar; the host stacks the 8 scalars into the final [8] vector.
"""

import numpy as np

B = 8
NH = 30
J = 17
H0 = W0 = 256
H1 = W1 = 512
N0 = J * H0 * W0
N1 = J * H1 * W1
NTOT = N0 + N1
NR = 2 * J * NH           # 1020 gathered elements
NC = 8                    # chunks of 128 (r = c*128 + p)
NI = 128 * NC
BIG = 1.0e9               # pad avg rows 30/31 -> exp(-BIG^2/2) = 0

_CACHE = {}

# ---------------------------------------------------------------------------
# host-side constants: chunk layout r = c*128 + p, f = r // NH, nh = r % NH
# ---------------------------------------------------------------------------


def _host_constants():
    if "consts" in _CACHE:
        return _CACHE["consts"]
    r = np.arange(NI)
    valid = r < NR
    f = np.where(valid, r // NH, 0)
    nh = np.where(valid, r % NH, 0)
    lvl = f // J
    j = f % J
    wmul = np.where(valid, np.where(lvl == 0, W0, W1), 0)
    base = np.where(valid, np.where(lvl == 0, j * H0 * W0, N0 + j * H1 * W1), 0)

    def chunkify(a):  # [NI] -> [128, NC]
        return np.ascontiguousarray(a.reshape(NC, 128).T)

    kcw = chunkify(wmul).astype(np.int32)
    kcb = chunkify(base).astype(np.int32)
    L0 = chunkify((valid & (lvl == 0)).astype(np.float32))
    L1 = chunkify((valid & (lvl == 1)).astype(np.float32))
    E = np.zeros((128, NC * NH), dtype=np.float32)
    for c in range(NC):
        rr = np.arange(c * 128, (c + 1) * 128)
        ok = rr < NR
        E[ok, c * NH + (rr[ok] % NH)] = 1.0
    cf = np.concatenate([L0, L1, E], axis=1).astype(np.float32)
    col_x = (lvl * 3 * J + 3 * j).astype(np.int64)
    _CACHE["consts"] = dict(
        kcw=kcw, kcb=kcb, cf=cf, nh=nh, col_x=col_x, valid=valid,
        chunkify=chunkify,
    )
    return _CACHE["consts"]


def make_in_maps(tag_maps0, tag_maps1, kps0, kps1):
    tag_maps0 = np.asarray(tag_maps0, dtype=np.float32)
    tag_maps1 = np.asarray(tag_maps1, dtype=np.float32)
    kps0 = np.asarray(kps0, dtype=np.int32)
    kps1 = np.asarray(kps1, dtype=np.int32)
    C = _host_constants()
    nh, col_x, valid = C["nh"], C["col_x"], C["valid"]
    chunkify = C["chunkify"]
    in_maps = []
    for b in range(B):
        tm = np.concatenate(
            [tag_maps0[b].ravel(), tag_maps1[b].ravel()]
        ).reshape(NTOT, 1)
        kp = np.concatenate([kps0[b], kps1[b]], axis=1)  # [30, 102]
        xs = np.zeros(NI, np.int32)
        ys = np.zeros(NI, np.int32)
        vs = np.zeros(NI, np.int32)
        xs[valid] = kp[nh[valid], col_x[valid]]
        ys[valid] = kp[nh[valid], col_x[valid] + 1]
        vs[valid] = kp[nh[valid], col_x[valid] + 2]
        kpg = np.stack(
            [chunkify(xs), chunkify(ys), chunkify(vs)], axis=2
        ).reshape(128, 3 * NC)
        ki = np.concatenate([kpg, C["kcw"], C["kcb"]], axis=1)  # [128, 40]
        in_maps.append({"tm": tm, "kp": kp, "ki": ki, "cf": C["cf"]})
    return in_maps


# ---------------------------------------------------------------------------
# device kernel
# ---------------------------------------------------------------------------


def _build_nc():
    from concourse import bacc, mybir
    import concourse.tile as tile
    from concourse.bass import IndirectOffsetOnAxis

    f32 = mybir.dt.float32
    i32 = mybir.dt.int32
    Alu = mybir.AluOpType
    X = mybir.AxisListType.X

    nc = bacc.Bacc()
    TM = nc.declare_dram_parameter("tm", [NTOT, 1], f32, isOutput=False)
    KP = nc.declare_dram_parameter("kp", [NH, 6 * J], i32, isOutput=False)
    KI = nc.declare_dram_parameter("ki", [128, 5 * NC], i32, isOutput=False)
    CF = nc.declare_dram_parameter(
        "cf", [128, 2 * NC + NC * NH], f32, isOutput=False
    )
    OUT = nc.declare_dram_parameter("out", [1, 1], f32, isOutput=True)

    with tile.TileContext(nc) as tc:
        with (
            tc.tile_pool(name="sb", bufs=1) as sb,
            tc.tile_pool(name="pp", bufs=1, space="PSUM") as pp,
        ):
            kt = sb.tile([NH, 6 * J], i32)
            ki = sb.tile([128, 5 * NC], i32)
            cf = sb.tile([128, 2 * NC + NC * NH], f32)
            idxc = sb.tile([128, NC], i32)
            S = sb.tile([128, NC], f32)
            maskg = sb.tile([128, NC], f32)
            T = sb.tile([128, 4 * NC], f32)
            sgq = sb.tile([128, 2 * NC], f32)
            maskf = sb.tile([NH, 2 * J], f32)
            cnt = sb.tile([NH, 2], f32)
            den = sb.tile([NH, 2], f32)
            rden = sb.tile([NH, 2], f32)
            st = sb.tile([NH, 4], f32)
            avg0 = sb.tile([NH, 2], f32)
            u = sb.tile([NH, 2], f32)
            avg32 = sb.tile([32, 2], f32)
            avgsrc = sb.tile([32, 64], f32)
            avgT = sb.tile([32, 64], f32)
            d2 = sb.tile([NH, 64], f32)
            pm = sb.tile([NH, 64], f32)
            pack = sb.tile([NH, 6], f32)
            ones = sb.tile([NH, 1], f32)
            warm = sb.tile([1, 1], f32)
            sums = sb.tile([1, 6], f32)
            rec = sb.tile([1, 6], f32)
            m1 = sb.tile([1, 4], f32)
            res = sb.tile([1, 1], f32)
            ps_st = pp.tile([NH, 4], f32)
            ps_f = pp.tile([1, 6], f32)

            # Warm the ACT Exp table during startup.
            nc.vector.memset(warm[:], 0.0)
            nc.scalar.activation(
                warm[:], warm[:], mybir.ActivationFunctionType.Exp
            )

            # Inputs in; ki first (it gates the gathers).
            nc.sync.dma_start(ki[:], KI[:])
            nc.gpsimd.dma_start(kt[:], KP[:])
            nc.gpsimd.dma_start(cf[:], CF[:])

            # Gather indices in chunk layout: idx = x*W + y + base.
            xg = ki[:, 0 : 3 * NC : 3]
            yg = ki[:, 1 : 3 * NC : 3]
            vg = ki[:, 2 : 3 * NC : 3]
            wmv = ki[:, 3 * NC : 4 * NC]
            bsv = ki[:, 4 * NC : 5 * NC]
            nc.vector.tensor_tensor(
                out=idxc[:, 0:1], in0=xg[:, 0:1], in1=wmv[:, 0:1], op=Alu.mult
            )
            nc.vector.tensor_tensor(
                out=idxc[:, 0:1], in0=idxc[:, 0:1], in1=yg[:, 0:1], op=Alu.add
            )
            nc.vector.tensor_tensor(
                out=idxc[:, 0:1], in0=idxc[:, 0:1], in1=bsv[:, 0:1], op=Alu.add
            )
            nc.vector.tensor_tensor(
                out=idxc[:, 1:NC], in0=xg[:, 1:NC], in1=wmv[:, 1:NC], op=Alu.mult
            )
            nc.vector.tensor_tensor(
                out=idxc[:, 1:NC], in0=idxc[:, 1:NC], in1=yg[:, 1:NC], op=Alu.add
            )
            nc.vector.tensor_tensor(
                out=idxc[:, 1:NC], in0=idxc[:, 1:NC], in1=bsv[:, 1:NC], op=Alu.add
            )
            nc.vector.tensor_scalar(
                out=maskg[:], in0=vg, scalar1=0, scalar2=None, op0=Alu.is_gt
            )

            # The only touch of the big tag maps: 1020 scalars in 8 indirect
            # DMAs; per-chunk stat products + matmuls pipeline right behind
            # each chunk's gather.
            L0 = cf[:, 0:NC]
            L1 = cf[:, NC : 2 * NC]
            for c in range(NC):
                nc.gpsimd.indirect_dma_start(
                    out=S[:, c : c + 1],
                    out_offset=None,
                    in_=TM[:],
                    in_offset=IndirectOffsetOnAxis(ap=idxc[:, c : c + 1], axis=0),
                )
                cs = slice(2 * c, 2 * c + 1)
                cs2 = slice(2 * c + 1, 2 * c + 2)
                nc.vector.tensor_tensor(
                    out=sgq[:, cs], in0=S[:, c : c + 1], in1=maskg[:, c : c + 1],
                    op=Alu.mult,
                )
                nc.vector.tensor_tensor(
                    out=sgq[:, cs2], in0=sgq[:, cs], in1=S[:, c : c + 1],
                    op=Alu.mult,
                )
                # T[:, {c, 8+c}] = sg * [L0_c, L1_c];
                # T[:, {16+c, 24+c}] = sg2 * [L0_c, L1_c]
                lc = cf[:, c : 2 * NC : NC]
                nc.vector.tensor_tensor(
                    out=T[:, c : NC + c + 1 : NC],
                    in0=sgq[:, cs].to_broadcast([128, 2]),
                    in1=lc,
                    op=Alu.mult,
                )
                nc.vector.tensor_tensor(
                    out=T[:, 2 * NC + c : 3 * NC + c + 1 : NC],
                    in0=sgq[:, cs2].to_broadcast([128, 2]),
                    in1=lc,
                    op=Alu.mult,
                )
                nc.tensor.matmul(
                    ps_st[:],
                    lhsT=cf[:, 2 * NC + c * NH : 2 * NC + (c + 1) * NH],
                    rhs=T[:, c : 4 * NC : NC],
                    start=(c == 0),
                    stop=(c == NC - 1),
                )

            # Joint-count path (independent of gathers; fills DVE idle time).
            vis = kt[:, 2 : 6 * J : 3]
            nc.vector.tensor_scalar(
                out=maskf[:], in0=vis, scalar1=0, scalar2=None, op0=Alu.is_gt
            )
            nc.vector.reduce_sum(
                out=cnt[:], in_=maskf[:].rearrange("p (l j) -> p l j", l=2), axis=X
            )
            nc.vector.tensor_scalar(
                out=den[:], in0=cnt[:], scalar1=1.0, scalar2=None, op0=Alu.max
            )
            nc.vector.reciprocal(rden[:], den[:])
            nc.vector.tensor_scalar(
                out=pack[:, 4:6], in0=cnt[:], scalar1=0.0, scalar2=None, op0=Alu.is_gt
            )
            nc.vector.memset(avg32[:], BIG)
            nc.vector.memset(ones[:], 1.0)

            # Per-human stats -> averages, pull (read stats from PSUM).
            sv = ps_st[:, 0:2]
            s2 = ps_st[:, 2:4]
            nc.vector.tensor_tensor(out=avg0[:], in0=sv, in1=rden[:], op=Alu.mult)
            nc.vector.tensor_tensor(
                out=avg32[0:NH, :], in0=avg0[:], in1=pack[:, 4:6], op=Alu.mult
            )
            # pull = s2 - sv*avg0 (zero when cnt == 0 since sv = s2 = 0)
            nc.vector.tensor_tensor(out=u[:], in0=sv, in1=avg0[:], op=Alu.mult)
            nc.vector.tensor_tensor(
                out=pack[:, 0:4:2], in0=s2, in1=u[:], op=Alu.subtract
            )

            # Push: pairwise means via 32x32 block stream transpose.
            nc.vector.tensor_copy(
                out=avgsrc[:].rearrange("p (l j) -> p l j", l=2),
                in_=avg32[:].to_broadcast([32, 2, 32]),
            )
            nc.vector.transpose(avgT[:], avgsrc[:])
            nc.vector.tensor_tensor(
                out=d2[:].rearrange("p (l j) -> p l j", l=2),
                in0=avgT[0:NH, :].rearrange("p (l j) -> p l j", l=2),
                in1=avg32[0:NH, :].to_broadcast([NH, 2, 32]),
                op=Alu.subtract,
            )
            nc.vector.tensor_tensor(out=d2[:], in0=d2[:], in1=d2[:], op=Alu.mult)
            nc.scalar.activation(
                pm[:, 0:32], d2[:, 0:32], mybir.ActivationFunctionType.Exp,
                scale=-0.5, accum_out=pack[:, 1:2],
            )
            nc.scalar.activation(
                pm[:, 32:64], d2[:, 32:64], mybir.ActivationFunctionType.Exp,
                scale=-0.5, accum_out=pack[:, 3:4],
            )

            # Column sums over the 30 humans via PE, then the final scalar.
            nc.tensor.matmul(
                ps_f[:], lhsT=ones[:], rhs=pack[:], start=True, stop=True
            )
            nc.vector.reciprocal(rec[:], ps_f[:])
            # m1 = [P0*rn0, Q0*rn0, P1*rn1, Q1*rn1] in one paired-AP mult
            nc.vector.tensor_tensor(
                out=m1[:].rearrange("p (l q) -> p l q", l=2),
                in0=ps_f[:, 0:4].rearrange("p (l q) -> p l q", l=2),
                in1=rec[:, 4:6].to_broadcast([1, 2, 2]),
                op=Alu.mult,
            )
            # push terms get the second /nh
            nc.vector.tensor_tensor(
                out=m1[:, 1:4:2], in0=m1[:, 1:4:2], in1=rec[:, 4:6], op=Alu.mult
            )
            nc.vector.reduce_sum(out=res[:], in_=m1[:], axis=X)

            nc.sync.dma_start(OUT[:], res[:])

    nc.finalize()
    return nc


def _get_nc():
    if "nc" not in _CACHE:
        _CACHE["nc"] = _build_nc()
    return _CACHE["nc"]


def kernel(tag_maps0, tag_maps1, kps0, kps1):
    from concourse.bass_utils import run_bass_kernel_spmd

    nc = _get_nc()
    in_maps = make_in_maps(tag_maps0, tag_maps1, kps0, kps1)
    out = run_bass_kernel_spmd(nc, in_maps, core_ids=list(range(B)))
    return np.array(
        [np.asarray(out.results[b]["out"]).reshape(()) for b in range(B)],
        dtype=np.float32,
    )
